# revision 24
# baseline (speedup 1.0000x reference)
"""MixLoRA-MoE Trainium2 kernel: 8-core data-parallel over tokens.

Math restructuring vs the reference scan:
  final = sum_e w_e * (silu(g_e) * u_e) @ Wd.T + lora_down terms
        = hbar @ Wd.T + sum_e (wh_e @ Ad_e.T) @ (2 Bd_e.T)   [linearity]
  where wh_e = w_e * silu(g_e) * u_e and hbar = sum_e wh_e.
So the expensive down projection through Wd runs ONCE on the weighted
combination instead of once per expert.

On-chip layout is transposed ([feature_part, token_free]) so every matmul
operand streams from DRAM in its natural (pre-transposed on host) layout.
All heavy matmuls run with bf16 operands (1 cycle/row on the PE, FWL weight
loads, half the f32 weight DMA); the router keeps exact-f32 logits for the
top-2 selection.  Per-expert LoRA deltas are applied directly into the base
PSUM accumulation via a host-packed +/- pair tensor (expert e's matmul adds
delta_e and subtracts delta_{e-1}, so the base never needs re-computation or
copies).  The f-loop is software-pipelined: tile f+1's independent base
matmuls are interleaved ahead of tile f's semaphore-gated delta matmuls in
the PE FIFO, and the lora-down-A matmuls are issued as col-tiled batches.
The lora-down-B contraction is a single matmul over all experts' stacked
rank rows.
"""
import os
import sys
import types

sys.path.insert(0, '/opt/trn_rl_repo')

import numpy as np
import ml_dtypes

BF16NP = ml_dtypes.bfloat16

# --- optional NTFF profiling shim (trace support under axon) ---
try:
    import antenv
    if 'antenv.axon_hooks' not in sys.modules:
        _m = types.ModuleType('antenv.axon_hooks')
        _hook_store = {}
        _m.set_axon_ntff_profile_hook = lambda h: _hook_store.__setitem__('h', h)
        _m.get_axon_ntff_profile_hook = lambda: _hook_store.get('h')
        sys.modules['antenv.axon_hooks'] = _m
        antenv.axon_hooks = _m
        try:
            from trn_agent_boot.trn_boot import _ntff_profile_via_ctypes
            _hook = _ntff_profile_via_ctypes('/opt/axon/libaxon_pjrt.so')
            if _hook is not None:
                _m.set_axon_ntff_profile_hook(_hook)
        except Exception:
            pass
except Exception:
    pass

import concourse.bass as bass
import concourse.mybir as mybir
from concourse import bacc
from concourse.tile import TileContext
from concourse import bass_utils

F32 = mybir.dt.float32
F32R = mybir.dt.float32r
BF16 = mybir.dt.bfloat16
AF = mybir.ActivationFunctionType
ALU = mybir.AluOpType

P = 128
D = 2048          # d_model
F = 8192          # d_ff
E = 8             # experts
R = 16            # lora rank
NCORES = 8
T_FULL = 4096
TC = T_FULL // NCORES   # 512 tokens per core
DKT = D // P            # 16 k-tiles over d_model
FT = F // P             # 64 f-tiles over d_ff
DT_TILES = D // P       # 16 output d-tiles

LAST_RESULT = {}        # test harness introspection (exec_time_ns etc.)
_NC_CACHE = {}


def build_nc():
    if 'nc' in _NC_CACHE:
        return _NC_CACHE['nc']
    nc = bacc.Bacc(None, target_bir_lowering=False)

    # ---- DRAM I/O ----
    xt_d = nc.dram_tensor("xt", [D, TC], F32, kind="ExternalInput")
    xtb_d = nc.dram_tensor("xtb", [D, TC], BF16, kind="ExternalInput")
    wgt_d = nc.dram_tensor("wgt", [D, F], BF16, kind="ExternalInput")
    wut_d = nc.dram_tensor("wut", [D, F], BF16, kind="ExternalInput")
    wdt_d = nc.dram_tensor("wdt", [F, D], BF16, kind="ExternalInput")
    rwt_d = nc.dram_tensor("rwt", [D, E], F32, kind="ExternalInput")
    agp_d = nc.dram_tensor("agp", [D, E * R], BF16, kind="ExternalInput")
    aup_d = nc.dram_tensor("aup", [D, E * R], BF16, kind="ExternalInput")
    pmw_d = nc.dram_tensor("pmw", [64, E, F], BF16, kind="ExternalInput")
    adt_d = nc.dram_tensor("adt", [F, E, 32], BF16, kind="ExternalInput")
    bd2_d = nc.dram_tensor("bd2", [E * R, D], BF16, kind="ExternalInput")
    oneh_d = nc.dram_tensor("oneh", [E, E, P], F32R, kind="ExternalInput")
    idt_d = nc.dram_tensor("idt", [P, P], F32, kind="ExternalInput")
    out_d = nc.dram_tensor("outT", [D, TC], F32, kind="ExternalOutput")

    with TileContext(nc) as tc:
        with tc.tile_pool(name="big", bufs=1) as big, \
             tc.tile_pool(name="wstream", bufs=4) as wstream, \
             tc.tile_pool(name="pmstream", bufs=3) as pmstream, \
             tc.tile_pool(name="adtp", bufs=2) as adtp, \
             tc.tile_pool(name="ebuf", bufs=4) as ebuf, \
             tc.tile_pool(name="xbuf", bufs=2) as xbuf, \
             tc.tile_pool(name="whb", bufs=12) as whb, \
             tc.tile_pool(name="obuf", bufs=2) as obuf, \
             tc.tile_pool(name="pp", bufs=3, space="PSUM") as pp, \
             tc.tile_pool(name="ppk", bufs=1, space="PSUM") as ppk:

            # ---- persistent SBUF ----
            xtr = big.tile([P, DKT, TC], BF16, name="xtr")
            hbar = big.tile([P, FT, TC], BF16, name="hbar")
            wb = big.tile([P, E, TC], F32, name="wb")
            spm = big.tile([64, E, TC], BF16, name="spm")
            psb = big.tile([P, TC], BF16, name="psb")
            w8 = big.tile([E, 4, P], F32R, name="w8")
            rw = big.tile([P, DKT, E], F32, name="rw")
            oneh = big.tile([E, E, P], F32R, name="oneh")
            idt = big.tile([P, P], F32, name="idt")
            scr = big.tile([P, 16], F32, name="scr")
            wtl = big.tile([P, 4, E], F32, name="wtl")

            # persistent PSUM: p accumulators (2 banks, 4 experts each via col groups)
            pbank = [ppk.tile([P, TC], F32, name=f"pbank{i}") for i in range(2)]

            # DMA order is tuned so the PE can start tile 0's base matmuls
            # ~5us in: bf16 x first (host-prepared), then f0 weights, then the
            # router's exact-f32 x, then lora tensors.
            nc.sync.dma_start(xtr, xtb_d.rearrange("(kt p) t -> p kt t", p=P))

            adtl = [None] * (FT // 4 + 1)

            def load_adt(f):
                adt_t = adtp.tile([P, 4, E, 32], BF16, name="adt")
                nc.sync.dma_start(
                    adt_t, adt_d[f * P:(f + 4) * P, :, :].rearrange(
                        "(fo p) e r -> p fo e r", p=P))
                adtl[f // 4] = adt_t

            def start_tile(f, with_adt=True):
                wg_t = wstream.tile([P, DKT, P], BF16, name="wchunk")
                nc.sync.dma_start(wg_t, wgt_d[:, bass.ts(f, P)].rearrange(
                    "(kt p) m -> p kt m", p=P))
                wu_t = wstream.tile([P, DKT, P], BF16, name="wchunk")
                nc.sync.dma_start(wu_t, wut_d[:, bass.ts(f, P)].rearrange(
                    "(kt p) m -> p kt m", p=P))
                pm_t = pmstream.tile([64, E, P], BF16, name="pmt")
                nc.sync.dma_start(pm_t, pmw_d[:, :, bass.ts(f, P)])
                if with_adt and f % 4 == 0:
                    load_adt(f)
                bg = pp.tile([P, TC], F32, name="bankg")
                bu = pp.tile([P, TC], F32, name="banku")
                return {"wg": wg_t, "wu": wu_t, "pm": pm_t, "bg": bg, "bu": bu,
                        "kt": 0}

            def pump_base(st, n):
                # issue up to n of tile st's 32 base matmuls (g first, then u)
                for _ in range(n):
                    k = st["kt"]
                    if k < DKT:
                        nc.tensor.matmul(st["bg"], st["wg"][:, k, :],
                                         xtr[:, k, :], start=(k == 0), stop=False)
                    elif k < 2 * DKT:
                        kk = k - DKT
                        nc.tensor.matmul(st["bu"], st["wu"][:, kk, :],
                                         xtr[:, kk, :], start=(kk == 0), stop=False)
                    else:
                        return
                    st["kt"] += 1

            cur = start_tile(0, with_adt=False)
            pump_base(cur, 2 * DKT)

            nc.sync.dma_start(rw, rwt_d.rearrange("(kt p) e -> p kt e", p=P))
            nc.sync.dma_start(oneh, oneh_d[:, :, :])
            nc.sync.dma_start(idt, idt_d[:, :])

            # ================= router =================
            # x at full f32 in a short-lived pool: the router's top-2 selection
            # needs exact f32 logits (rounded copies flip near-ties).
            for tt in range(4):
                xt_tt = xbuf.tile([P, DKT, P], F32, name="xchunk")
                nc.sync.dma_start(
                    xt_tt, xt_d[:, bass.ts(tt, P)].rearrange("(kt p) t -> p kt t", p=P))
                psl = pp.tile([P, TC], F32, name="bankg")
                for kt in range(DKT):
                    # full-f32 matmul: near-tie top-2 selection must match the
                    # reference's f32 logits (f32r rounding flips ~2 tokens)
                    nc.tensor.matmul(psl[:, 0:E], xt_tt[:, kt, :],
                                     rw[:, kt, :], start=(kt == 0),
                                     stop=(kt == DKT - 1))
                # top-2 selection happens on raw logits (exp is monotonic but the
                # ACT exp LUT has enough error to flip ~1e-4 near-ties)
                nmx = scr[:, 1:2]
                mx = scr[:, 0:1]
                m2 = scr[:, 2:3]
                rcp = scr[:, 3:4]
                z = scr[:, 4:12]
                lcp = wtl[:, 0, :]
                nc.vector.tensor_reduce(nmx, psl[:, 0:E], axis=mybir.AxisListType.X,
                                        op=ALU.max, negate=True)
                nc.vector.tensor_scalar_mul(mx, nmx, -1.0)
                nc.scalar.activation(z, psl[:, 0:E], AF.Exp, bias=nmx)
                # logits with the max knocked out: l - 1e30*(l >= max)
                lm1 = wtl[:, 1, :]
                nc.vector.tensor_scalar(lm1, psl[:, 0:E], mx, -1e30,
                                        op0=ALU.is_ge, op1=ALU.mult)
                nc.vector.tensor_tensor(lcp, psl[:, 0:E], lm1, op=ALU.add)
                nc.vector.tensor_reduce(m2, lcp, axis=mybir.AxisListType.X, op=ALU.max)
                # select z where logit >= second max; normalize by selected sum
                wsel = wtl[:, 1, :]
                nc.vector.scalar_tensor_tensor(wsel, psl[:, 0:E], m2, z,
                                               op0=ALU.is_ge, op1=ALU.mult)
                nc.vector.tensor_reduce(rcp, wsel, axis=mybir.AxisListType.X, op=ALU.add)
                nc.vector.reciprocal(rcp, rcp)
                wcur = wtl[:, 2 + (tt % 2), :]
                nc.vector.tensor_scalar_mul(wcur, wsel, rcp)
                # transpose [128t, 8e] -> psum [8e, 128t]
                psw = pp.tile([P, TC], F32, name="banku")
                nc.tensor.transpose(psw[0:E, 0:P], wcur, idt)
                nc.vector.tensor_copy(w8[:, tt, :], psw[0:E, 0:P])
            # broadcast rows of w8 -> wb[128, e, TC] via one-hot matmuls
            w8flat = w8.rearrange("p a b -> p (a b)")
            for e in range(E):
                pswb = pp.tile([P, TC], F32, name="bankg")
                nc.tensor.matmul(pswb, oneh[:, e, :], w8flat, start=True, stop=True)
                nc.vector.tensor_copy(wb[:, e, :], pswb)

            # ================= lora-A projections (s) =================
            for gi, src in enumerate((agp_d, aup_d)):
                ap_t = wstream.tile([P, DKT, P], BF16, name="wchunk")
                nc.sync.dma_start(ap_t, src.rearrange("(kt p) m -> p kt m", p=P))
                sps = pp.tile([P, TC], F32, name="bankg")
                for kt in range(DKT):
                    nc.tensor.matmul(sps, ap_t[:, kt, :], xtr[:, kt, :],
                                     start=(kt == 0), stop=(kt == DKT - 1))
                s_stage = ebuf.tile([P, TC], BF16, name="t1")
                nc.vector.tensor_copy(s_stage, sps)
                # scatter into +/- pair layout rows: [prev(16) | cur(16)] per expert
                base = 32 * gi
                nc.gpsimd.memset(spm[base:base + R, 0, :], 0.0)
                for e in range(E):
                    if e >= 1:
                        nc.sync.dma_start(spm[base:base + R, e, :],
                                          s_stage[(e - 1) * R:e * R, :])
                    nc.sync.dma_start(spm[base + R:base + 2 * R, e, :],
                                      s_stage[e * R:(e + 1) * R, :])

            # ================= main f-loop (software-pipelined) =================
            # Tile f's expert chain is gated by ACT/VEC latencies (the in-psum
            # morph serializes delta_e behind silu_{e-1}).  The PE queue is
            # FIFO, so tile f+1's independent base matmuls are interleaved into
            # the chain's program order to keep the PE busy during those waits.
            load_adt(0)
            for f in range(FT):
                nxt = start_tile(f + 1) if f + 1 < FT else None
                adt_t = adtl[f // 4]
                whs = []
                for e in range(E):
                    # independent base matmuls for tile f+1 go ahead of the
                    # dependent delta in the PE FIFO: the delta's semaphore
                    # wait then overlaps real work instead of blocking it
                    if nxt:
                        pump_base(nxt, 3)
                    # apply lora delta_e (and remove delta_{e-1}) in-psum
                    nc.tensor.matmul(cur["bg"], cur["pm"][0:32, e, :],
                                     spm[0:32, e, :], start=False,
                                     stop=(e == E - 1))
                    nc.tensor.matmul(cur["bu"], cur["pm"][32:64, e, :],
                                     spm[32:64, e, :], start=False,
                                     stop=(e == E - 1))
                    s_act = ebuf.tile([P, TC], BF16, name="sact")
                    nc.scalar.activation(s_act, cur["bg"], AF.Silu)
                    t1 = ebuf.tile([P, TC], BF16, name="t1")
                    nc.vector.scalar_tensor_tensor(t1, cur["bu"], 1.0, s_act,
                                                   op0=ALU.bypass, op1=ALU.mult)
                    wh = whb.tile([P, TC], BF16, name="wh")
                    nc.gpsimd.tensor_tensor(wh, t1, wb[:, e, :], op=ALU.mult)
                    whs.append(wh)
                # lora-down A batches: 4 back-to-back col-tiled matmuls per
                # psum bank so adjacent col-groups can overlap in the array
                for b in range(2):
                    for j in range(4):
                        eg = 4 * b + j
                        nc.tensor.matmul(pbank[b][32 * j:32 * j + 32, :],
                                         adt_t[:, f % 4, eg, :], whs[eg],
                                         tile_position=(0, 32 * j),
                                         start=(f == 0), stop=(f == FT - 1))
                    if nxt:
                        pump_base(nxt, 4)
                nc.vector.tensor_tensor(hbar[:, f, :], whs[0], whs[1], op=ALU.add)
                for e2 in range(2, E):
                    nc.vector.tensor_tensor(hbar[:, f, :], hbar[:, f, :],
                                            whs[e2], op=ALU.add)
                cur = nxt

            # extract weighted p to sbuf: stack all experts' rank rows on
            # 128 partitions so the lora-down-B matmul is a single contraction
            for b in range(2):
                p_stage = ebuf.tile([P, TC], BF16, name="t1")
                nc.vector.tensor_copy(p_stage, pbank[b])
                for eo in range(4):
                    e = b * 4 + eo
                    nc.sync.dma_start(psb[e * R:(e + 1) * R, :],
                                      p_stage[32 * eo:32 * eo + R, :])

            # ================= down projection =================
            for d in range(DT_TILES):
                psd = pp.tile([P, TC], F32, name="bankg")
                for fc in range(4):
                    wd_t = wstream.tile([P, DKT, P], BF16, name="wdchunk")
                    nc.sync.dma_start(
                        wd_t, wdt_d[fc * 2048:(fc + 1) * 2048, bass.ts(d, P)].rearrange(
                            "(kt p) m -> p kt m", p=P))
                    for kt in range(DKT):
                        nc.tensor.matmul(psd, wd_t[:, kt, :], hbar[:, fc * DKT + kt, :],
                                         start=(fc == 0 and kt == 0), stop=False)
                bd2_t = pmstream.tile([P, P], BF16, name="bd2")
                nc.sync.dma_start(bd2_t, bd2_d[:, bass.ts(d, P)])
                nc.tensor.matmul(psd, bd2_t, psb, start=False, stop=True)
                o_t = obuf.tile([P, TC], F32, name="osb")
                nc.scalar.activation(o_t, psd, AF.Copy)
                nc.sync.dma_start(out_d[bass.ts(d, P), :], o_t)

    nc.finalize()
    _NC_CACHE['nc'] = nc
    return nc


def _host_prep(hidden_states, router_w, Wg, Wu, Wd, Ag, Bg, Au, Bu, Ad, Bd):
    f32 = np.float32
    X = np.ascontiguousarray(hidden_states.reshape(T_FULL, D), dtype=f32)
    xT = np.ascontiguousarray(X.T)
    shared = {
        "wgt": np.ascontiguousarray(Wg.T).astype(BF16NP),
        "wut": np.ascontiguousarray(Wu.T).astype(BF16NP),
        "wdt": np.ascontiguousarray(Wd.T).astype(BF16NP),
        "rwt": np.ascontiguousarray(router_w.T, dtype=f32),
        "agp": np.ascontiguousarray(Ag.transpose(2, 0, 1).reshape(D, E * R)).astype(BF16NP),
        "aup": np.ascontiguousarray(Au.transpose(2, 0, 1).reshape(D, E * R)).astype(BF16NP),
    }
    # +/- pair tensor: rows 0:16 gate-prev(-), 16:32 gate-cur(+), 32:48 up-prev(-), 48:64 up-cur(+)
    pmw = np.zeros((64, E, F), dtype=f32)
    BgT = np.transpose(Bg, (0, 2, 1))  # [E, R, F]
    BuT = np.transpose(Bu, (0, 2, 1))
    for e in range(E):
        if e >= 1:
            pmw[0:R, e] = -2.0 * BgT[e - 1]
            pmw[32:48, e] = -2.0 * BuT[e - 1]
        pmw[R:32, e] = 2.0 * BgT[e]
        pmw[48:64, e] = 2.0 * BuT[e]
    shared["pmw"] = pmw.astype(BF16NP)
    adt = np.zeros((F, E, 32), dtype=f32)
    adt[:, :, 0:R] = Ad.transpose(2, 0, 1)
    shared["adt"] = adt.astype(BF16NP)
    shared["bd2"] = np.ascontiguousarray(
        (2.0 * Bd.transpose(0, 2, 1)).reshape(E * R, D)).astype(BF16NP)
    oneh = np.zeros((E, E, P), dtype=f32)
    for e in range(E):
        oneh[e, e, :] = 1.0
    shared["oneh"] = oneh
    shared["idt"] = np.eye(P, dtype=f32)
    in_maps = []
    for c in range(NCORES):
        m = dict(shared)
        m["xt"] = np.ascontiguousarray(xT[:, c * TC:(c + 1) * TC])
        m["xtb"] = m["xt"].astype(BF16NP)
        in_maps.append(m)
    return in_maps


def kernel(hidden_states, router_w, Wg, Wu, Wd, Ag, Bg, Au, Bu, Ad, Bd):
    hidden_states = np.asarray(hidden_states)
    nc = build_nc()
    in_maps = _host_prep(np.asarray(hidden_states, dtype=np.float32),
                         np.asarray(router_w), np.asarray(Wg), np.asarray(Wu),
                         np.asarray(Wd), np.asarray(Ag), np.asarray(Bg),
                         np.asarray(Au), np.asarray(Bu), np.asarray(Ad),
                         np.asarray(Bd))
    trace = bool(os.environ.get("TRNK_TRACE"))
    res = bass_utils.run_bass_kernel_spmd(
        nc, in_maps, core_ids=list(range(NCORES)), trace=trace)
    LAST_RESULT['exec_time_ns'] = res.exec_time_ns
    LAST_RESULT['res'] = res
    out = np.empty((T_FULL, D), dtype=np.float32)
    for c in range(NCORES):
        out[c * TC:(c + 1) * TC, :] = res.results[c]["outT"].T
    return out.reshape(hidden_states.shape[0], hidden_states.shape[1], D)

